# revision 1
# baseline (speedup 1.0000x reference)
"""Trainium2 Bass kernel for nn_Cycle_Consistency_Loss (soft-DTW-style
cycle loss). Self-contained: host-side packing + SPMD Bass program on 8
NeuronCores + host reduction.

Math (per pair (a,b), both directions; x = seq[q], y = seq[k], lens = src_len//4):
  alpha = softmax_j(-|x_i-y_j|^2) over valid j -> snn = alpha @ y
  beta  = softmax_k(-|snn_i-x_k|^2) over valid k
  u = E_beta[k], std = E_beta[(k-u)^2]
  li = (i-u)^2/std + 0.005*ln(std), summed over valid i; total / n_pairs.

Kernel decomposition: work items = 512-query blocks of each direction.
Per item, scores are computed transposed ([keys->partitions, queries->free])
via augmented matmuls so softmax denominators reduce over partitions on the
PE (no running max needed: pass-A scores <= 0; pass-B scores bounded).
Variance uses a two-round pass B (u first, then sum P2*(u-k)^2 elementwise)
to avoid catastrophic cancellation. Items are sorted by size and dealt
8-at-a-time into steps; loop bounds are compile-time per step.
"""
import sys
import numpy as np

sys.path.insert(0, "/opt/trn_rl_repo")

QB = 512          # query block = matmul free dim = one PSUM bank of fp32
KG = 256          # key group (2 chunks of 128 partitions)
NCORES = 8
PENALTY = 0.01
BIG = 1.0e30
STD_FLOOR = 1.0e-35


def _ceil(a, b):
    return -(-a // b)


class _Item:
    __slots__ = ("qi", "ki", "Lq", "Lk", "qb", "ga", "gb", "dummy")

    def __init__(self, qi, ki, Lq, Lk, qb):
        self.qi, self.ki, self.Lq, self.Lk, self.qb = qi, ki, Lq, Lk, qb
        self.ga = _ceil(Lk, KG)
        self.gb = _ceil(Lq, KG)
        self.dummy = False


class _Dummy:
    qi = ki = Lq = Lk = qb = 0
    ga = gb = 0
    dummy = True


def pack(seq, src_len, combinations):
    """Build the step plan and per-core input arrays.

    Per-core inputs (all fp32):
      kA  [34, CA]   pass-A key operand rows [yT; y2; 1] (masked keys y2=BIG)
      vAr [128, CA//128*33]  pass-A values, pre-swizzled so the on-chip
                     [128, 2GA, 33] tile loads with contiguous per-partition
                     rows: vAr[p, g*33+d] = vA[g*128+p, d], vA = [y | 1]
      qA  [34, QB*NS] pass-A query operand rows [2xT; -1; -x2]
      kB  [33, CB]   pass-B key operand rows [2xT; x2] (masked keys x2=BIG)
      kvo [128, 66]  col 2j = global key index of chunk j (j<32), col 2j+1 = 1;
                     col 64 = ones (sum-weights lhsT), col 65 = 0
      qidx/qmask [128, 4*NS] absolute query index / valid mask per B-slot
    """
    seq = np.asarray(seq, np.float32)
    lens = (np.asarray(src_len).astype(np.int64) // 4).astype(np.int64)
    comb = np.asarray(combinations).astype(np.int64)

    items = []
    for a, b in comb:
        for qi, ki in ((a, b), (b, a)):
            Lq, Lk = int(lens[qi]), int(lens[ki])
            if Lq <= 0 or Lk <= 0:
                continue
            for qb in range(_ceil(Lq, QB)):
                items.append(_Item(int(qi), int(ki), Lq, Lk, qb))
    items.sort(key=lambda it: -(it.ga + it.gb))
    NS = max(1, _ceil(len(items), NCORES))
    while len(items) < NS * NCORES:
        items.append(_Dummy())

    GA = [max(max(items[s * NCORES + c].ga for c in range(NCORES)), 1)
          for s in range(NS)]
    GB = [max(max(items[s * NCORES + c].gb for c in range(NCORES)), 1)
          for s in range(NS)]
    CA = sum(GA) * KG
    CB = sum(GB) * KG

    sq2 = np.einsum("btd,btd->bt", seq, seq).astype(np.float32)

    kvo = np.zeros((128, 66), np.float32)
    for j in range(32):
        kvo[:, 2 * j] = (j * 128 + np.arange(128)).astype(np.float32)
        kvo[:, 2 * j + 1] = 1.0
    kvo[:, 64] = 1.0

    cores = []
    for c in range(NCORES):
        kA = np.zeros((34, CA), np.float32)
        vA = np.zeros((CA, 33), np.float32)
        qA = np.zeros((34, QB * NS), np.float32)
        kB = np.zeros((33, CB), np.float32)
        qidx = np.zeros((128, 4 * NS), np.float32)
        qmask = np.zeros((128, 4 * NS), np.float32)
        offa = 0
        offb = 0
        its = []
        for s in range(NS):
            it = items[s * NCORES + c]
            its.append(it)
            na = GA[s] * KG
            nb = GB[s] * KG
            ka = kA[:, offa:offa + na]
            va = vA[offa:offa + na]
            kb = kB[:, offb:offb + nb]
            qa = qA[:, s * QB:(s + 1) * QB]
            if it.dummy:
                ka[33, :] = 1.0
                va[:, 32] = 1.0
            else:
                y = seq[it.ki]
                x = seq[it.qi]
                Lk, Lq = it.Lk, it.Lq
                nk = min(Lk, na)
                ka[0:32, :nk] = y[:nk].T
                ka[32, :nk] = sq2[it.ki, :nk]
                ka[33, :nk] = 1.0
                ka[32, nk:] = BIG
                ka[33, nk:] = 1.0
                va[:nk, 0:32] = y[:nk]
                va[:nk, 32] = 1.0
                q0 = it.qb * QB
                nq = min(Lq - q0, QB)
                qa[0:32, :nq] = 2.0 * x[q0:q0 + nq].T
                qa[32, :nq] = -1.0
                qa[33, :nq] = -sq2[it.qi, q0:q0 + nq]
                nkb = min(Lq, nb)
                kb[0:32, :nkb] = 2.0 * x[:nkb].T
                kb[32, :nkb] = sq2[it.qi, :nkb]
                kb[32, nkb:] = BIG
                for c4 in range(4):
                    ii = q0 + c4 * 128 + np.arange(128)
                    qidx[:, s * 4 + c4] = ii.astype(np.float32)
                    qmask[:, s * 4 + c4] = (ii < Lq).astype(np.float32)
            offa += na
            offb += nb
        vAr = np.ascontiguousarray(
            vA.reshape(CA // 128, 128, 33).transpose(1, 0, 2).reshape(128, -1))
        cores.append(dict(kA=kA, vAr=vAr, qA=qA, kB=kB, kvo=kvo,
                          qidx=qidx, qmask=qmask, items=its))
    plan = dict(NS=NS, GA=GA, GB=GB, CA=CA, CB=CB)
    return plan, cores


def build_program(plan):
    """Build the SPMD Bass program for the given step plan."""
    import concourse.bass as bass
    import concourse.bacc as bacc
    import concourse.mybir as mybir
    import concourse.tile as tile

    F32 = mybir.dt.float32
    AFT = mybir.ActivationFunctionType
    NS, GA, GB = plan["NS"], plan["GA"], plan["GB"]
    CA, CB = plan["CA"], plan["CB"]
    GBmax = max(GB)
    GAmax = max(GA)

    nc = bacc.Bacc("TRN2", target_bir_lowering=False, debug=False,
                   num_devices=NCORES)
    kA_d = nc.dram_tensor("kA", [34, CA], F32, kind="ExternalInput")
    vAr_d = nc.dram_tensor("vAr", [128, (CA // 128) * 33], F32,
                           kind="ExternalInput")
    qA_d = nc.dram_tensor("qA", [34, QB * NS], F32, kind="ExternalInput")
    kB_d = nc.dram_tensor("kB", [33, CB], F32, kind="ExternalInput")
    kvo_d = nc.dram_tensor("kvo", [128, 66], F32, kind="ExternalInput")
    qidx_d = nc.dram_tensor("qidx", [128, 4 * NS], F32, kind="ExternalInput")
    qmask_d = nc.dram_tensor("qmask", [128, 4 * NS], F32, kind="ExternalInput")
    out_d = nc.dram_tensor("out", [1, 1], F32, kind="ExternalOutput")

    with tile.TileContext(nc) as tc:
        with (
            tc.tile_pool(name="keys", bufs=2) as keys_pool,
            tc.tile_pool(name="vals", bufs=2) as vals_pool,
            tc.tile_pool(name="qrys", bufs=2) as qrys_pool,
            tc.tile_pool(name="pa", bufs=2) as pa_pool,
            tc.tile_pool(name="cache", bufs=1) as cache_pool,
            tc.tile_pool(name="epi", bufs=1) as epi_pool,
            tc.tile_pool(name="b2", bufs=2) as b2_pool,
            tc.tile_pool(name="fin", bufs=1) as fin_pool,
            tc.tile_pool(name="sc_ps", bufs=2, space="PSUM") as sc_psum,
            tc.tile_pool(name="na_ps", bufs=1, space="PSUM") as na_psum,
            tc.tile_pool(name="t_ps", bufs=1, space="PSUM") as t_psum,
            tc.tile_pool(name="sd_ps", bufs=1, space="PSUM") as sd_psum,
        ):
            kvo = fin_pool.tile([128, 66], F32)
            nc.sync.dma_start(kvo[:], kvo_d[:])
            qidx = fin_pool.tile([128, 4 * NS], F32)
            nc.sync.dma_start(qidx[:], qidx_d[:])
            qmask = fin_pool.tile([128, 4 * NS], F32)
            nc.sync.dma_start(qmask[:], qmask_d[:])
            stats_u = fin_pool.tile([128, 4 * NS], F32)
            stats_s = fin_pool.tile([128, 4 * NS], F32)

            offa = 0
            offb = 0
            for s in range(NS):
                ga, gb = GA[s], GB[s]
                na, nb = ga * KG, gb * KG
                # ---- load this step's operands
                kA_t = keys_pool.tile([34, GAmax * KG], F32, tag="kA")
                nc.sync.dma_start(kA_t[:, :na], kA_d[:, offa:offa + na])
                vA_t = vals_pool.tile([128, GAmax * 2 * 33], F32, tag="vA")
                nc.sync.dma_start(
                    vA_t[:, :ga * 66],
                    vAr_d[:, (offa // 128) * 33:((offa + na) // 128) * 33])
                qA_t = qrys_pool.tile([34, QB], F32, tag="qA")
                nc.sync.dma_start(qA_t[:], qA_d[:, s * QB:(s + 1) * QB])
                kB_t = keys_pool.tile([33, GBmax * KG], F32, tag="kB")
                nc.sync.dma_start(kB_t[:, :nb], kB_d[:, offb:offb + nb])

                # ---- pass A: numA[0:32] = snn.T * Z, numA[32] = Z
                numA = na_psum.tile([33, QB], F32)
                for g in range(ga):
                    sc = sc_psum.tile([128, 2 * QB], F32, tag="sc")
                    P = pa_pool.tile([128, 2 * QB], F32, tag="pa")
                    for h in range(2):
                        ch = 2 * g + h
                        nc.tensor.matmul(
                            sc[:, h * QB:(h + 1) * QB],
                            kA_t[:, ch * 128:(ch + 1) * 128], qA_t[:],
                            start=True, stop=True)
                    nc.scalar.activation(P[:], sc[:], AFT.Exp)
                    for h in range(2):
                        ch = 2 * g + h
                        nc.tensor.matmul(
                            numA[:],
                            vA_t[:, ch * 33:(ch + 1) * 33],
                            P[:, h * QB:(h + 1) * QB],
                            start=(g == 0 and h == 0),
                            stop=(g == ga - 1 and h == 1))

                # ---- epilogue A: R2 = [snn.T; -1]
                nsb = epi_pool.tile([33, QB], F32, tag="nsb")
                nc.vector.tensor_copy(nsb[:], numA[:])
                zrow = epi_pool.tile([1, QB], F32, tag="zrow")
                nc.sync.dma_start(zrow[:], nsb[32:33, :])
                rz0 = epi_pool.tile([1, QB], F32, tag="rz0")
                nc.vector.reciprocal(rz0[:], zrow[:])
                rb = epi_pool.tile([32, QB], F32, tag="rb")
                nc.gpsimd.partition_broadcast(rb[:], rz0[:])
                R2 = epi_pool.tile([33, QB], F32, tag="R2")
                nc.gpsimd.memset(R2[32:33, :], -1.0)
                nc.vector.tensor_mul(R2[0:32, :], nsb[0:32, :], rb[:])

                # ---- pass B1: P2 cached; T = [r0; Z2]
                cache = cache_pool.tile([128, GBmax * 2 * QB], F32, tag="p2c")
                T = t_psum.tile([2, QB], F32, tag="T")
                for g in range(gb):
                    sc = sc_psum.tile([128, 2 * QB], F32, tag="sc")
                    for h in range(2):
                        ch = 2 * g + h
                        nc.tensor.matmul(
                            sc[:, h * QB:(h + 1) * QB],
                            kB_t[:, ch * 128:(ch + 1) * 128], R2[:],
                            start=True, stop=True)
                    nc.scalar.activation(
                        cache[:, g * 2 * QB:(g + 1) * 2 * QB], sc[:], AFT.Exp)
                    for h in range(2):
                        ch = 2 * g + h
                        nc.tensor.matmul(
                            T[:],
                            kvo[:, 2 * ch:2 * ch + 2],
                            cache[:, (2 * g + h) * QB:(2 * g + h + 1) * QB],
                            start=(g == 0 and h == 0),
                            stop=(g == gb - 1 and h == 1))

                # ---- mid: u = r0 / Z2, broadcast
                tt = epi_pool.tile([2, QB], F32, tag="tt")
                nc.vector.tensor_copy(tt[:], T[:])
                z2row = epi_pool.tile([1, QB], F32, tag="z2row")
                nc.sync.dma_start(z2row[:], tt[1:2, :])
                rz2 = epi_pool.tile([1, QB], F32, tag="rz2")
                nc.vector.reciprocal(rz2[:], z2row[:])
                u0 = epi_pool.tile([1, QB], F32, tag="u0")
                nc.vector.tensor_mul(u0[:], tt[0:1, :], rz2[:])
                ub = epi_pool.tile([128, QB], F32, tag="ub")
                nc.gpsimd.partition_broadcast(ub[:], u0[:])

                # ---- pass B2: stdsum = sum_k P2 * (u-k)^2
                stdsum = sd_psum.tile([1, QB], F32, tag="sd")
                for g in range(gb):
                    for h in range(2):
                        ch = 2 * g + h
                        d = b2_pool.tile([128, QB], F32, tag="d")
                        nc.vector.tensor_scalar_sub(
                            d[:], ub[:], kvo[:, 2 * ch:2 * ch + 1])
                        sq = b2_pool.tile([128, QB], F32, tag="sq")
                        nc.vector.tensor_mul(sq[:], d[:], d[:])
                        w = b2_pool.tile([128, QB], F32, tag="w")
                        nc.gpsimd.tensor_mul(
                            w[:], sq[:],
                            cache[:, (2 * g + h) * QB:(2 * g + h + 1) * QB])
                        nc.tensor.matmul(
                            stdsum[:], kvo[:, 64:65], w[:],
                            start=(g == 0 and h == 0),
                            stop=(g == gb - 1 and h == 1))

                # ---- epilogue B: write u, std into stats via transpose-DMA
                sstd = epi_pool.tile([1, QB], F32, tag="sstd")
                nc.vector.tensor_mul(sstd[:], stdsum[:], rz2[:])
                for c4 in range(4):
                    nc.sync.dma_start(
                        stats_u[:, s * 4 + c4:s * 4 + c4 + 1],
                        u0[0:1, c4 * 128:(c4 + 1) * 128])
                    nc.sync.dma_start(
                        stats_s[:, s * 4 + c4:s * 4 + c4 + 1],
                        sstd[0:1, c4 * 128:(c4 + 1) * 128])
                offa += na
                offb += nb

            # ---- final: li = (i-u)^2/std + 0.005*ln(std), masked sum
            W = 4 * NS
            stdc = fin_pool.tile([128, W], F32)
            nc.vector.tensor_scalar_max(stdc[:], stats_s[:], STD_FLOOR)
            rstd = fin_pool.tile([128, W], F32)
            nc.vector.reciprocal(rstd[:], stdc[:])
            delta = fin_pool.tile([128, W], F32)
            nc.vector.tensor_sub(delta[:], qidx[:], stats_u[:])
            d2 = fin_pool.tile([128, W], F32)
            nc.vector.tensor_mul(d2[:], delta[:], delta[:])
            t1 = fin_pool.tile([128, W], F32)
            nc.vector.tensor_mul(t1[:], d2[:], rstd[:])
            lg = fin_pool.tile([128, W], F32)
            nc.scalar.activation(lg[:], stdc[:], AFT.Ln)
            lgs = fin_pool.tile([128, W], F32)
            nc.vector.tensor_scalar_mul(lgs[:], lg[:], 0.5 * PENALTY)
            li = fin_pool.tile([128, W], F32)
            nc.vector.tensor_add(li[:], t1[:], lgs[:])
            lim = fin_pool.tile([128, W], F32)
            nc.vector.tensor_mul(lim[:], li[:], qmask[:])
            rowsum = fin_pool.tile([128, 1], F32)
            nc.vector.reduce_sum(rowsum[:], lim[:],
                                 axis=mybir.AxisListType.X)
            tot = t_psum.tile([1, 1], F32, tag="tot")
            nc.tensor.matmul(tot[:], rowsum[:], kvo[0:128, 64:65],
                             start=True, stop=True)
            osb = fin_pool.tile([1, 1], F32)
            nc.vector.tensor_copy(osb[:], tot[:])
            nc.sync.dma_start(out_d[:], osb[:])

    nc.compile()
    return nc


def kernel(seq, src_len, combinations):
    from concourse.bass_utils import run_bass_kernel_spmd

    plan, cores = pack(seq, src_len, combinations)
    nc = build_program(plan)
    in_maps = [
        {k: ci[k] for k in
         ("kA", "vAr", "qA", "kB", "kvo", "qidx", "qmask")}
        for ci in cores
    ]
    res = run_bass_kernel_spmd(nc, in_maps, list(range(NCORES)))
    tot = np.float32(0.0)
    for c in range(NCORES):
        tot += np.float32(res.results[c]["out"][0, 0])
    n_pairs = np.asarray(combinations).shape[0]
    return np.float32(tot / np.float32(n_pairs))



# revision 6
# speedup vs baseline: 2.4148x; 2.4148x over previous
"""Trainium2 Bass kernel for nn_Cycle_Consistency_Loss (soft-DTW-style
cycle loss). Self-contained: host-side packing + SPMD Bass program on 8
NeuronCores + host reduction.

Math (per pair (a,b), both directions; x = seq[q], y = seq[k], lens = src_len//4):
  alpha = softmax_j(-|x_i-y_j|^2) over valid j -> snn = alpha @ y
  beta  = softmax_k(-|snn_i-x_k|^2) over valid k
  u = E_beta[k], std = E_beta[(k-u)^2]
  li = (i-u)^2/std + 0.005*ln(std), summed over valid i; total / n_pairs.

Kernel decomposition: work items = 512-query blocks of each direction.
Per item, scores are computed transposed ([keys->partitions, queries->free])
via augmented matmuls so softmax denominators reduce over partitions on the
PE. Matmuls run in float32r (full-rate fp32). Pass B computes, per
128-key chunk m, chunk-centered index moments [Zc, M1c, M2c] directly on
the PE using a sparse constant operand (rows land at 3m..3m+2 of a [96,512]
PSUM accumulator); the variance is then recombined per chunk center
(avoids E[k^2]-u^2 cancellation) with a handful of vector ops per block.
Items are packed into octets by (ga,gb) type to minimize per-step padding.
"""
import sys
import numpy as np

sys.path.insert(0, "/opt/trn_rl_repo")

QB = 512          # query block = matmul free dim = one PSUM bank of fp32
KG = 256          # key group (2 chunks of 128 partitions)
NCORES = 8
PENALTY = 0.01
BIG = 1.0e30
STD_FLOOR = 1.0e-6
MM_F32R = True    # use float32r (full-rate) matmuls


def _ceil(a, b):
    return -(-a // b)


class _Item:
    __slots__ = ("qi", "ki", "Lq", "Lk", "qb", "ga", "gb", "dummy")

    def __init__(self, qi, ki, Lq, Lk, qb):
        self.qi, self.ki, self.Lq, self.Lk, self.qb = qi, ki, Lq, Lk, qb
        self.ga = _ceil(Lk, KG)
        self.gb = _ceil(Lq, KG)
        self.dummy = False


class _Dummy:
    qi = ki = Lq = Lk = qb = 0
    ga = gb = 0
    dummy = True


def _pack_octets(items):
    """Pack items into octets (one per step, one item per core) minimizing
    sum over steps of (max ga + max gb)."""
    from collections import defaultdict

    groups = defaultdict(list)
    for it in items:
        groups[(it.ga, it.gb)].append(it)
    octets = []
    rest = []
    for key in sorted(groups, key=lambda k: -(k[0] + k[1])):
        lst = groups[key]
        while len(lst) >= NCORES:
            octets.append(lst[:NCORES])
            lst = lst[NCORES:]
        rest.extend(lst)
    rest.sort(key=lambda it: -(it.ga + it.gb))
    while rest:
        cur = [rest.pop(0)]
        cga, cgb = cur[0].ga, cur[0].gb
        while len(cur) < NCORES and rest:
            best_i, best_key = None, None
            for i, it in enumerate(rest):
                inc = max(it.ga - cga, 0) + max(it.gb - cgb, 0)
                key = (inc, -(it.ga + it.gb))
                if best_key is None or key < best_key:
                    best_key, best_i = key, i
            it = rest.pop(best_i)
            cur.append(it)
            cga = max(cga, it.ga)
            cgb = max(cgb, it.gb)
        octets.append(cur)
    for o in octets:
        while len(o) < NCORES:
            o.append(_Dummy())

    def cost(o):
        return (max(max(it.ga for it in o), 1)
                + max(max(it.gb for it in o), 1))

    for _ in range(40):
        improved = False
        for i in range(len(octets)):
            for j in range(i + 1, len(octets)):
                oi, oj = octets[i], octets[j]
                c0 = cost(oi) + cost(oj)
                for a in range(NCORES):
                    for b in range(NCORES):
                        oi[a], oj[b] = oj[b], oi[a]
                        c1 = cost(oi) + cost(oj)
                        if c1 < c0:
                            c0 = c1
                            improved = True
                        else:
                            oi[a], oj[b] = oj[b], oi[a]
        if not improved:
            break
    octets.sort(key=cost, reverse=True)
    return octets


def pack(seq, src_len, combinations):
    """Build the step plan and per-core input arrays.

    Per-core inputs (all fp32):
      kA  [34, CA]   pass-A key operand rows [yT; y2; 1] (masked keys y2=BIG)
      vAr [128, CA//128*33]  pass-A values, pre-swizzled so the on-chip
                     [128, 2GA, 33] tile loads with contiguous per-partition
                     rows: vAr[p, g*33+d] = vA[g*128+p, d], vA = [y | 1]
      qA  [34, QB*NS] pass-A query operand rows [2xT; -1; -x2]
      kB  [33, CB]   pass-B key operand rows [2xT; x2] (masked keys x2=BIG)
      mql [128, 32*96] sparse moment lhsT: block m has cols [3m,3m+1,3m+2]
                     = [1, r, r^2], r = p-63.5; all other cols 0
      wz  [96, 4]    col0: 1 on rows 3m (-> Z); col1: c_m on 3m, 1 on 3m+1
                     (-> S1); col2: ones on first 96 rows (stdZ reduce);
                     col3: 0
      coef [96, 4]   col0 c_m (chunk centers, all 3 rows), col1 a2 (1 on 3m),
                     col2 a1 (-2 on 3m+1), col3 a0 (1 on 3m+2)
      ones [128, 1]
      qidx/qmask [128, 4*NS] absolute query index / valid mask per B-slot
    """
    seq = np.asarray(seq, np.float32)
    lens = (np.asarray(src_len).astype(np.int64) // 4).astype(np.int64)
    comb = np.asarray(combinations).astype(np.int64)

    items = []
    for a, b in comb:
        for qi, ki in ((a, b), (b, a)):
            Lq, Lk = int(lens[qi]), int(lens[ki])
            if Lq <= 0 or Lk <= 0:
                continue
            for qb in range(_ceil(Lq, QB)):
                items.append(_Item(int(qi), int(ki), Lq, Lk, qb))

    octets = _pack_octets(items)
    NS = len(octets)
    GA = [max(max(it.ga for it in o), 1) for o in octets]
    GB = [max(max(it.gb for it in o), 1) for o in octets]
    CA = sum(GA) * KG
    CB = sum(GB) * KG

    sq2 = np.einsum("btd,btd->bt", seq, seq).astype(np.float32)

    # constants (same for all cores)
    p = np.arange(128, dtype=np.float32)
    r = p - 63.5
    mql = np.zeros((128, 32 * 96), np.float32)
    for m in range(32):
        mql[:, m * 96 + 3 * m + 0] = 1.0
        mql[:, m * 96 + 3 * m + 1] = r
        mql[:, m * 96 + 3 * m + 2] = r * r
    cm = 128.0 * np.arange(32, dtype=np.float32) + 63.5
    # wz: col0 -> Z at out partition 0; col32 -> S1 at out partition 32;
    # col33 -> ones for the stdZ reduce (separate matmul, out partition 64)
    wz = np.zeros((96, 34), np.float32)
    coef = np.zeros((96, 4), np.float32)
    for m in range(32):
        wz[3 * m + 0, 0] = 1.0
        wz[3 * m + 0, 32] = cm[m]
        wz[3 * m + 1, 32] = 1.0
        coef[3 * m:3 * m + 3, 0] = cm[m]
        coef[3 * m + 0, 1] = 1.0
        coef[3 * m + 1, 2] = -2.0
        coef[3 * m + 2, 3] = 1.0
    wz[:, 33] = 1.0
    ones = np.ones((128, 1), np.float32)
    neg1 = np.full((1, QB), -1.0, np.float32)

    cores = []
    for c in range(NCORES):
        kA = np.zeros((34, CA), np.float32)
        vA = np.zeros((CA, 33), np.float32)
        qA = np.zeros((34, QB * NS), np.float32)
        kB = np.zeros((33, CB), np.float32)
        qidx = np.zeros((128, 4 * NS), np.float32)
        qmask = np.zeros((128, 4 * NS), np.float32)
        offa = 0
        offb = 0
        its = []
        for s in range(NS):
            it = octets[s][c]
            its.append(it)
            na = GA[s] * KG
            nb = GB[s] * KG
            ka = kA[:, offa:offa + na]
            va = vA[offa:offa + na]
            kb = kB[:, offb:offb + nb]
            qa = qA[:, s * QB:(s + 1) * QB]
            if it.dummy:
                ka[33, :] = 1.0
                va[:, 32] = 1.0
            else:
                y = seq[it.ki]
                x = seq[it.qi]
                Lk, Lq = it.Lk, it.Lq
                nk = min(Lk, na)
                ka[0:32, :nk] = y[:nk].T
                ka[32, :nk] = sq2[it.ki, :nk]
                ka[33, :nk] = 1.0
                ka[32, nk:] = BIG
                ka[33, nk:] = 1.0
                va[:nk, 0:32] = y[:nk]
                va[:nk, 32] = 1.0
                q0 = it.qb * QB
                nq = min(Lq - q0, QB)
                qa[0:32, :nq] = 2.0 * x[q0:q0 + nq].T
                qa[32, :nq] = -1.0
                qa[33, :nq] = -sq2[it.qi, q0:q0 + nq]
                nkb = min(Lq, nb)
                kb[0:32, :nkb] = 2.0 * x[:nkb].T
                kb[32, :nkb] = sq2[it.qi, :nkb]
                kb[32, nkb:] = BIG
                for c4 in range(4):
                    ii = q0 + c4 * 128 + np.arange(128)
                    qidx[:, s * 4 + c4] = ii.astype(np.float32)
                    qmask[:, s * 4 + c4] = (ii < Lq).astype(np.float32)
            offa += na
            offb += nb
        vAr = np.ascontiguousarray(
            vA.reshape(CA // 128, 128, 33).transpose(1, 0, 2).reshape(128, -1))
        cores.append(dict(kA=kA, vAr=vAr, qA=qA, kB=kB, mql=mql, wz=wz,
                          coef=coef, ones=ones, neg1=neg1, qidx=qidx,
                          qmask=qmask, items=its))
    plan = dict(NS=NS, GA=GA, GB=GB, CA=CA, CB=CB)
    return plan, cores


IN_KEYS = ("kA", "vAr", "qA", "kB", "mql", "wz", "coef", "ones", "neg1",
           "qidx", "qmask")


def build_program(plan):
    """Build the SPMD Bass program for the given step plan."""
    import concourse.bass as bass
    import concourse.bacc as bacc
    import concourse.mybir as mybir
    import concourse.tile as tile
    from concourse.alu_op_type import AluOpType

    F32 = mybir.dt.float32
    F32R = mybir.dt.float32r if MM_F32R else mybir.dt.float32
    AFT = mybir.ActivationFunctionType
    NS, GA, GB = plan["NS"], plan["GA"], plan["GB"]
    CA, CB = plan["CA"], plan["CB"]
    GBmax = max(GB)
    GAmax = max(GA)

    def mr(ap):
        return ap

    nc = bacc.Bacc("TRN2", target_bir_lowering=False, debug=False,
                   num_devices=NCORES)
    kA_d = nc.dram_tensor("kA", [34, CA], F32R, kind="ExternalInput")
    vAr_d = nc.dram_tensor("vAr", [128, (CA // 128) * 33], F32R,
                           kind="ExternalInput")
    qA_d = nc.dram_tensor("qA", [34, QB * NS], F32R, kind="ExternalInput")
    kB_d = nc.dram_tensor("kB", [33, CB], F32R, kind="ExternalInput")
    mql_d = nc.dram_tensor("mql", [128, 32 * 96], F32R, kind="ExternalInput")
    wz_d = nc.dram_tensor("wz", [96, 34], F32, kind="ExternalInput")
    coef_d = nc.dram_tensor("coef", [96, 4], F32, kind="ExternalInput")
    ones_d = nc.dram_tensor("ones", [128, 1], F32, kind="ExternalInput")
    neg1_d = nc.dram_tensor("neg1", [1, QB], F32R, kind="ExternalInput")
    qidx_d = nc.dram_tensor("qidx", [128, 4 * NS], F32, kind="ExternalInput")
    qmask_d = nc.dram_tensor("qmask", [128, 4 * NS], F32,
                             kind="ExternalInput")
    out_d = nc.dram_tensor("out", [1, 1], F32, kind="ExternalOutput")

    with tile.TileContext(nc) as tc:
        with (
            tc.tile_pool(name="keys", bufs=2) as keys_pool,
            tc.tile_pool(name="vals", bufs=2) as vals_pool,
            tc.tile_pool(name="qrys", bufs=2) as qrys_pool,
            tc.tile_pool(name="pa", bufs=2) as pa_pool,
            tc.tile_pool(name="epi", bufs=1) as epi_pool,
            tc.tile_pool(name="fin", bufs=1) as fin_pool,
            tc.tile_pool(name="sc_ps", bufs=2, space="PSUM") as sc_psum,
            tc.tile_pool(name="na_ps", bufs=1, space="PSUM") as na_psum,
            tc.tile_pool(name="mo_ps", bufs=1, space="PSUM") as mo_psum,
            tc.tile_pool(name="zs_ps", bufs=1, space="PSUM") as zs_psum,
        ):
            mql = fin_pool.tile([128, 32 * 96], F32R)
            nc.sync.dma_start(mql[:], mql_d[:])
            wz = fin_pool.tile([96, 34], F32)
            nc.sync.dma_start(wz[:], wz_d[:])
            coef = fin_pool.tile([96, 4], F32)
            nc.sync.dma_start(coef[:], coef_d[:])
            ones = fin_pool.tile([128, 1], F32)
            nc.sync.dma_start(ones[:], ones_d[:])
            qidx = fin_pool.tile([128, 4 * NS], F32)
            nc.sync.dma_start(qidx[:], qidx_d[:])
            qmask = fin_pool.tile([128, 4 * NS], F32)
            nc.sync.dma_start(qmask[:], qmask_d[:])
            stats_u = fin_pool.tile([128, 4 * NS], F32)
            stats_s = fin_pool.tile([128, 4 * NS], F32)
            R2 = fin_pool.tile([33, QB], F32R)
            nc.sync.dma_start(R2[32:33, :], neg1_d[:])

            offa = 0
            offb = 0
            for s in range(NS):
                ga, gb = GA[s], GB[s]
                na, nb = ga * KG, gb * KG
                # ---- load this step's operands
                kA_t = keys_pool.tile([34, GAmax * KG], F32R, tag="kA")
                nc.sync.dma_start(kA_t[:, :na], kA_d[:, offa:offa + na])
                vA_t = vals_pool.tile([128, GAmax * 2 * 33], F32R, tag="vA")
                nc.sync.dma_start(
                    vA_t[:, :ga * 66],
                    vAr_d[:, (offa // 128) * 33:((offa + na) // 128) * 33])
                qA_t = qrys_pool.tile([34, QB], F32R, tag="qA")
                nc.sync.dma_start(qA_t[:], qA_d[:, s * QB:(s + 1) * QB])
                kB_t = keys_pool.tile([33, GBmax * KG], F32R, tag="kB")
                nc.sync.dma_start(kB_t[:, :nb], kB_d[:, offb:offb + nb])

                # ---- pass A: numA[0:32] = snn.T * Z, numA[32] = Z
                numA = na_psum.tile([33, QB], F32)
                for g in range(ga):
                    sc = sc_psum.tile([128, 2 * QB], F32, tag="sc")
                    P = pa_pool.tile([128, 2 * QB], F32R, tag="pa")
                    for h in range(2):
                        ch = 2 * g + h
                        nc.tensor.matmul(
                            sc[:, h * QB:(h + 1) * QB],
                            mr(kA_t[:, ch * 128:(ch + 1) * 128]), mr(qA_t[:]),
                            start=True, stop=True)
                    nc.scalar.activation(P[:], sc[:], AFT.Exp)
                    for h in range(2):
                        ch = 2 * g + h
                        nc.tensor.matmul(
                            numA[:],
                            mr(vA_t[:, ch * 33:(ch + 1) * 33]),
                            mr(P[:, h * QB:(h + 1) * QB]),
                            start=(g == 0 and h == 0),
                            stop=(g == ga - 1 and h == 1))

                # ---- epilogue A: R2 = [snn.T; -1]
                nsb = epi_pool.tile([33, QB], F32, tag="nsb")
                nc.vector.tensor_copy(nsb[:], numA[:])
                zrow = epi_pool.tile([1, QB], F32, tag="zrow")
                nc.sync.dma_start(zrow[:], nsb[32:33, :])
                rz0 = epi_pool.tile([1, QB], F32, tag="rz0")
                nc.vector.reciprocal(rz0[:], zrow[:])
                rb = epi_pool.tile([32, QB], F32, tag="rb")
                nc.gpsimd.partition_broadcast(rb[:], rz0[:])
                nc.vector.tensor_mul(R2[0:32, :], nsb[0:32, :], rb[:])

                # ---- pass B: chunk-centered moments on the PE
                mom = mo_psum.tile([96, QB], F32)
                for g in range(gb):
                    sc = sc_psum.tile([128, 2 * QB], F32, tag="sc")
                    P2 = pa_pool.tile([128, 2 * QB], F32R, tag="pa")
                    for h in range(2):
                        ch = 2 * g + h
                        nc.tensor.matmul(
                            sc[:, h * QB:(h + 1) * QB],
                            mr(kB_t[:, ch * 128:(ch + 1) * 128]), mr(R2[:]),
                            start=True, stop=True)
                    nc.scalar.activation(P2[:], sc[:], AFT.Exp)
                    for h in range(2):
                        m = 2 * g + h
                        nc.tensor.matmul(
                            mom[:],
                            mr(mql[:, m * 96:(m + 1) * 96]),
                            mr(P2[:, h * QB:(h + 1) * QB]),
                            start=(g == 0 and h == 0),
                            stop=(g == gb - 1 and h == 1))

                # ---- epilogue B: u = S1/Z; stdZ via chunk-center recombine
                Vm = epi_pool.tile([96, QB], F32, tag="Vm")
                nc.vector.tensor_copy(Vm[:], mom[:])
                zs = zs_psum.tile([128, QB], F32, tag="zs")
                nc.tensor.matmul(zs[0:33, :], wz[:, 0:33], Vm[:],
                                 start=True, stop=True)
                rz2 = epi_pool.tile([1, QB], F32, tag="rz2")
                nc.vector.reciprocal(rz2[:], zs[0:1, :])
                u0 = epi_pool.tile([1, QB], F32, tag="u0")
                nc.vector.tensor_mul(u0[:], zs[32:33, :], rz2[:])
                ub = epi_pool.tile([96, QB], F32, tag="ub")
                nc.gpsimd.partition_broadcast(ub[:], u0[:])
                # d = u - c;  F = (a2*d + a1)*d + a0;  W = F .* Vm
                dt_ = epi_pool.tile([96, QB], F32, tag="dt")
                nc.vector.tensor_scalar_sub(dt_[:], ub[:], coef[:, 0:1])
                tt = epi_pool.tile([96, QB], F32, tag="tt")
                nc.vector.tensor_scalar(tt[:], dt_[:], coef[:, 1:2],
                                        coef[:, 2:3], AluOpType.mult,
                                        AluOpType.add)
                q2 = epi_pool.tile([96, QB], F32, tag="q2")
                nc.vector.tensor_mul(q2[:], tt[:], dt_[:])
                Wt = epi_pool.tile([96, QB], F32, tag="Wt")
                nc.vector.scalar_tensor_tensor(Wt[:], q2[:], coef[:, 3:4],
                                               Vm[:], AluOpType.add,
                                               AluOpType.mult)
                nc.tensor.matmul(zs[64:65, :], wz[0:96, 33:34], Wt[:],
                                 start=True, stop=True)
                sstd = epi_pool.tile([1, QB], F32, tag="sstd")
                nc.vector.tensor_mul(sstd[:], zs[64:65, :], rz2[:])
                for c4 in range(4):
                    nc.sync.dma_start(
                        stats_u[:, s * 4 + c4:s * 4 + c4 + 1],
                        u0[0:1, c4 * 128:(c4 + 1) * 128])
                    nc.sync.dma_start(
                        stats_s[:, s * 4 + c4:s * 4 + c4 + 1],
                        sstd[0:1, c4 * 128:(c4 + 1) * 128])
                offa += na
                offb += nb

            # ---- final: li = (i-u)^2/std + 0.005*ln(std), masked sum
            W = 4 * NS
            stdc = fin_pool.tile([128, W], F32)
            nc.vector.tensor_scalar_max(stdc[:], stats_s[:], STD_FLOOR)
            rstd = fin_pool.tile([128, W], F32)
            nc.vector.reciprocal(rstd[:], stdc[:])
            delta = fin_pool.tile([128, W], F32)
            nc.vector.tensor_sub(delta[:], qidx[:], stats_u[:])
            d2 = fin_pool.tile([128, W], F32)
            nc.vector.tensor_mul(d2[:], delta[:], delta[:])
            t1 = fin_pool.tile([128, W], F32)
            nc.vector.tensor_mul(t1[:], d2[:], rstd[:])
            lg = fin_pool.tile([128, W], F32)
            nc.scalar.activation(lg[:], stdc[:], AFT.Ln)
            lgs = fin_pool.tile([128, W], F32)
            nc.vector.tensor_scalar_mul(lgs[:], lg[:], 0.5 * PENALTY)
            li = fin_pool.tile([128, W], F32)
            nc.vector.tensor_add(li[:], t1[:], lgs[:])
            lim = fin_pool.tile([128, W], F32)
            nc.vector.tensor_mul(lim[:], li[:], qmask[:])
            rowsum = fin_pool.tile([128, 1], F32)
            nc.vector.reduce_sum(rowsum[:], lim[:],
                                 axis=mybir.AxisListType.X)
            tot = zs_psum.tile([128, QB], F32, tag="zs")
            nc.tensor.matmul(tot[0:1, 0:1], rowsum[:], ones[:],
                             start=True, stop=True)
            osb = fin_pool.tile([1, 1], F32)
            nc.vector.tensor_copy(osb[:], tot[0:1, 0:1])
            nc.sync.dma_start(out_d[:], osb[:])

    nc.compile()
    return nc


def kernel(seq, src_len, combinations):
    from concourse.bass_utils import run_bass_kernel_spmd

    plan, cores = pack(seq, src_len, combinations)
    nc = build_program(plan)
    in_maps = [{k: ci[k] for k in IN_KEYS} for ci in cores]
    res = run_bass_kernel_spmd(nc, in_maps, list(range(NCORES)))
    tot = np.float32(0.0)
    for c in range(NCORES):
        tot += np.float32(res.results[c]["out"][0, 0])
    n_pairs = np.asarray(combinations).shape[0]
    return np.float32(tot / np.float32(n_pairs))


# revision 8
# speedup vs baseline: 2.9878x; 1.2373x over previous
"""Trainium2 Bass kernel for nn_Cycle_Consistency_Loss (soft-DTW-style
cycle loss). Self-contained: host-side packing + SPMD Bass program on 8
NeuronCores + host reduction.

Math (per pair (a,b), both directions; x = seq[q], y = seq[k], lens = src_len//4):
  alpha = softmax_j(-|x_i-y_j|^2) over valid j -> snn = alpha @ y
  beta  = softmax_k(-|snn_i-x_k|^2) over valid k
  u = E_beta[k], std = E_beta[(k-u)^2]
  li = (i-u)^2/std + 0.005*ln(std), summed over valid i; total / n_pairs.

Kernel decomposition: work items = 512-query blocks of each direction.
Per item, scores are computed transposed ([keys->partitions, queries->free])
via augmented matmuls so softmax denominators reduce over partitions on the
PE. Hot matmuls run in float32r (full PE rate). Pass B computes, per
128-key chunk m, chunk-centered index moments [Zc, M1c, M2c] on the PE via
a sparse constant operand (zero-padded columns accumulate rows 3m..3m+2 of
one [97,512] PSUM tile; column 96 accumulates total Z for free); variance
is recombined per integer chunk center (avoids E[k^2]-u^2 cancellation).
The program is software-pipelined: pass A of step s+1 is issued before
pass B of step s, and the pass-B epilogue is staggered over the next two
iterations, so the PE never waits on the DVE/GpSimd epilogue chains.
Items are packed into octets by (ga,gb) type to minimize per-step padding.
"""
import sys
import numpy as np

sys.path.insert(0, "/opt/trn_rl_repo")

QB = 512          # query block = matmul free dim = one PSUM bank of fp32
KG = 256          # key group (2 chunks of 128 partitions)
NCORES = 8
PENALTY = 0.01
BIG = 1.0e30
STD_FLOOR = 1.0e-6


def _ceil(a, b):
    return -(-a // b)


class _Item:
    __slots__ = ("qi", "ki", "Lq", "Lk", "qb", "ga", "gb", "dummy")

    def __init__(self, qi, ki, Lq, Lk, qb):
        self.qi, self.ki, self.Lq, self.Lk, self.qb = qi, ki, Lq, Lk, qb
        self.ga = _ceil(Lk, KG)
        self.gb = _ceil(Lq, KG)
        self.dummy = False


class _Dummy:
    qi = ki = Lq = Lk = qb = 0
    ga = gb = 0
    dummy = True


def _pack_octets(items):
    """Pack items into octets (one per step, one item per core) minimizing
    sum over steps of (max ga + max gb)."""
    from collections import defaultdict

    groups = defaultdict(list)
    for it in items:
        groups[(it.ga, it.gb)].append(it)
    octets = []
    rest = []
    for key in sorted(groups, key=lambda k: -(k[0] + k[1])):
        lst = groups[key]
        while len(lst) >= NCORES:
            octets.append(lst[:NCORES])
            lst = lst[NCORES:]
        rest.extend(lst)
    rest.sort(key=lambda it: -(it.ga + it.gb))
    while rest:
        cur = [rest.pop(0)]
        cga, cgb = cur[0].ga, cur[0].gb
        while len(cur) < NCORES and rest:
            best_i, best_key = None, None
            for i, it in enumerate(rest):
                inc = max(it.ga - cga, 0) + max(it.gb - cgb, 0)
                key = (inc, -(it.ga + it.gb))
                if best_key is None or key < best_key:
                    best_key, best_i = key, i
            it = rest.pop(best_i)
            cur.append(it)
            cga = max(cga, it.ga)
            cgb = max(cgb, it.gb)
        octets.append(cur)
    for o in octets:
        while len(o) < NCORES:
            o.append(_Dummy())

    def cost(o):
        return (max(max(it.ga for it in o), 1)
                + max(max(it.gb for it in o), 1))

    for _ in range(40):
        improved = False
        for i in range(len(octets)):
            for j in range(i + 1, len(octets)):
                oi, oj = octets[i], octets[j]
                c0 = cost(oi) + cost(oj)
                for a in range(NCORES):
                    for b in range(NCORES):
                        oi[a], oj[b] = oj[b], oi[a]
                        c1 = cost(oi) + cost(oj)
                        if c1 < c0:
                            c0 = c1
                            improved = True
                        else:
                            oi[a], oj[b] = oj[b], oi[a]
        if not improved:
            break
    octets.sort(key=cost, reverse=True)
    return octets


def pack(seq, src_len, combinations):
    """Build the step plan and per-core input arrays.

    Per-core inputs (fp32 bytes; hot matmul operands declared float32r):
      kA  [34, CA]   pass-A key operand rows [yT; y2; 1] (masked keys y2=BIG)
      vAr [128, CA//128*33]  pass-A values, pre-swizzled so the on-chip
                     [128, 2GA, 33] tile loads with contiguous per-partition
                     rows: vAr[p, g*33+d] = vA[g*128+p, d], vA = [y | 1]
      qA  [34, QB*NS] pass-A query operand rows [2xT; -1; -x2]
      kB  [33, CB]   pass-B key operand rows [2xT; x2] (masked keys x2=BIG)
      mql [128, 32*97] sparse moment lhsT: block m has cols [3m,3m+1,3m+2]
                     = [1, r, r^2] (r = p-64, integer-exact), col 96 = 1
                     (accumulates total Z at out row 96); all other cols 0
      w1  [96, 2]    col0: S1 weights (c_m on rows 3m, 1 on rows 3m+1);
                     col1: ones (stdZ reduce)
      coef [96, 4]   col0 c_m (chunk centers, all 3 rows), col1 a2 (1 on 3m),
                     col2 a1 (-2 on 3m+1), col3 a0 (1 on 3m+2)
      ones [128, 1]; neg1 [1, QB]
      qidx/qmask [128, 4*NS] absolute query index / valid mask per B-slot
    """
    seq = np.asarray(seq, np.float32)
    lens = (np.asarray(src_len).astype(np.int64) // 4).astype(np.int64)
    comb = np.asarray(combinations).astype(np.int64)

    items = []
    for a, b in comb:
        for qi, ki in ((a, b), (b, a)):
            Lq, Lk = int(lens[qi]), int(lens[ki])
            if Lq <= 0 or Lk <= 0:
                continue
            for qb in range(_ceil(Lq, QB)):
                items.append(_Item(int(qi), int(ki), Lq, Lk, qb))

    octets = _pack_octets(items)
    NS = len(octets)
    GA = [max(max(it.ga for it in o), 1) for o in octets]
    GB = [max(max(it.gb for it in o), 1) for o in octets]
    CA = sum(GA) * KG
    CB = sum(GB) * KG

    sq2 = np.einsum("btd,btd->bt", seq, seq).astype(np.float32)

    # constants (same for all cores); integer chunk centers c_m = 128m + 64
    p = np.arange(128, dtype=np.float32)
    r = p - 64.0
    mql = np.zeros((128, 32 * 97), np.float32)
    for m in range(32):
        mql[:, m * 97 + 3 * m + 0] = 1.0
        mql[:, m * 97 + 3 * m + 1] = r
        mql[:, m * 97 + 3 * m + 2] = r * r
        mql[:, m * 97 + 96] = 1.0
    cm = 128.0 * np.arange(32, dtype=np.float32) + 64.0
    w1 = np.zeros((96, 2), np.float32)
    coef = np.zeros((96, 4), np.float32)
    for m in range(32):
        w1[3 * m + 0, 0] = cm[m]
        w1[3 * m + 1, 0] = 1.0
        coef[3 * m:3 * m + 3, 0] = cm[m]
        coef[3 * m + 0, 1] = 1.0
        coef[3 * m + 1, 2] = -2.0
        coef[3 * m + 2, 3] = 1.0
    w1[:, 1] = 1.0
    ones = np.ones((128, 1), np.float32)
    neg1 = np.full((1, QB), -1.0, np.float32)

    cores = []
    for c in range(NCORES):
        kA = np.zeros((34, CA), np.float32)
        vA = np.zeros((CA, 33), np.float32)
        qA = np.zeros((34, QB * NS), np.float32)
        kB = np.zeros((33, CB), np.float32)
        qidx = np.zeros((128, 4 * NS), np.float32)
        qmask = np.zeros((128, 4 * NS), np.float32)
        offa = 0
        offb = 0
        its = []
        for s in range(NS):
            it = octets[s][c]
            its.append(it)
            na = GA[s] * KG
            nb = GB[s] * KG
            ka = kA[:, offa:offa + na]
            va = vA[offa:offa + na]
            kb = kB[:, offb:offb + nb]
            qa = qA[:, s * QB:(s + 1) * QB]
            if it.dummy:
                ka[33, :] = 1.0
                va[:, 32] = 1.0
            else:
                y = seq[it.ki]
                x = seq[it.qi]
                Lk, Lq = it.Lk, it.Lq
                nk = min(Lk, na)
                ka[0:32, :nk] = y[:nk].T
                ka[32, :nk] = sq2[it.ki, :nk]
                ka[33, :nk] = 1.0
                ka[32, nk:] = BIG
                ka[33, nk:] = 1.0
                va[:nk, 0:32] = y[:nk]
                va[:nk, 32] = 1.0
                q0 = it.qb * QB
                nq = min(Lq - q0, QB)
                qa[0:32, :nq] = 2.0 * x[q0:q0 + nq].T
                qa[32, :nq] = -1.0
                qa[33, :nq] = -sq2[it.qi, q0:q0 + nq]
                nkb = min(Lq, nb)
                kb[0:32, :nkb] = 2.0 * x[:nkb].T
                kb[32, :nkb] = sq2[it.qi, :nkb]
                kb[32, nkb:] = BIG
                for c4 in range(4):
                    ii = q0 + c4 * 128 + np.arange(128)
                    qidx[:, s * 4 + c4] = ii.astype(np.float32)
                    qmask[:, s * 4 + c4] = (ii < Lq).astype(np.float32)
            offa += na
            offb += nb
        vAr = np.ascontiguousarray(
            vA.reshape(CA // 128, 128, 33).transpose(1, 0, 2).reshape(128, -1))
        cores.append(dict(kA=kA, vAr=vAr, qA=qA, kB=kB, mql=mql, w1=w1,
                          coef=coef, ones=ones, neg1=neg1, qidx=qidx,
                          qmask=qmask, items=its))
    plan = dict(NS=NS, GA=GA, GB=GB, CA=CA, CB=CB)
    return plan, cores


IN_KEYS = ("kA", "vAr", "qA", "kB", "mql", "w1", "coef", "ones", "neg1",
           "qidx", "qmask")


def build_program(plan):
    """Build the software-pipelined SPMD Bass program."""
    import concourse.bass as bass
    import concourse.bacc as bacc
    import concourse.mybir as mybir
    import concourse.tile as tile
    from concourse.alu_op_type import AluOpType

    F32 = mybir.dt.float32
    F32R = mybir.dt.float32r
    AFT = mybir.ActivationFunctionType
    NS, GA, GB = plan["NS"], plan["GA"], plan["GB"]
    CA, CB = plan["CA"], plan["CB"]
    GBmax = max(GB)
    GAmax = max(GA)

    nc = bacc.Bacc("TRN2", target_bir_lowering=False, debug=False,
                   num_devices=NCORES)
    kA_d = nc.dram_tensor("kA", [34, CA], F32R, kind="ExternalInput")
    vAr_d = nc.dram_tensor("vAr", [128, (CA // 128) * 33], F32R,
                           kind="ExternalInput")
    qA_d = nc.dram_tensor("qA", [34, QB * NS], F32R, kind="ExternalInput")
    kB_d = nc.dram_tensor("kB", [33, CB], F32R, kind="ExternalInput")
    mql_d = nc.dram_tensor("mql", [128, 32 * 97], F32R, kind="ExternalInput")
    w1_d = nc.dram_tensor("w1", [96, 2], F32, kind="ExternalInput")
    coef_d = nc.dram_tensor("coef", [96, 4], F32, kind="ExternalInput")
    ones_d = nc.dram_tensor("ones", [128, 1], F32, kind="ExternalInput")
    neg1_d = nc.dram_tensor("neg1", [1, QB], F32R, kind="ExternalInput")
    qidx_d = nc.dram_tensor("qidx", [128, 4 * NS], F32, kind="ExternalInput")
    qmask_d = nc.dram_tensor("qmask", [128, 4 * NS], F32,
                             kind="ExternalInput")
    out_d = nc.dram_tensor("out", [1, 1], F32, kind="ExternalOutput")

    offa = [0] * (NS + 1)
    offb = [0] * (NS + 1)
    for s in range(NS):
        offa[s + 1] = offa[s] + GA[s] * KG
        offb[s + 1] = offb[s] + GB[s] * KG

    with tile.TileContext(nc) as tc:
        with (
            tc.tile_pool(name="keys", bufs=2) as keys_pool,
            tc.tile_pool(name="vals", bufs=2) as vals_pool,
            tc.tile_pool(name="qrys", bufs=2) as qrys_pool,
            tc.tile_pool(name="pa", bufs=2) as pa_pool,
            tc.tile_pool(name="epi", bufs=2) as epi_pool,
            tc.tile_pool(name="fin", bufs=1) as fin_pool,
            tc.tile_pool(name="sc_ps", bufs=2, space="PSUM") as sc_psum,
            tc.tile_pool(name="na_ps", bufs=1, space="PSUM") as na_psum,
            tc.tile_pool(name="mo_ps", bufs=1, space="PSUM") as mo_psum,
            tc.tile_pool(name="sm_ps", bufs=2, space="PSUM") as sm_psum,
        ):
            mql = fin_pool.tile([128, 32 * 97], F32R)
            nc.sync.dma_start(mql[:], mql_d[:])
            w1 = fin_pool.tile([96, 2], F32)
            nc.sync.dma_start(w1[:], w1_d[:])
            coef = fin_pool.tile([96, 4], F32)
            nc.sync.dma_start(coef[:], coef_d[:])
            ones = fin_pool.tile([128, 1], F32)
            nc.sync.dma_start(ones[:], ones_d[:])
            qidx = fin_pool.tile([128, 4 * NS], F32)
            nc.sync.dma_start(qidx[:], qidx_d[:])
            qmask = fin_pool.tile([128, 4 * NS], F32)
            nc.sync.dma_start(qmask[:], qmask_d[:])
            stats_u = fin_pool.tile([128, 4 * NS], F32)
            stats_s = fin_pool.tile([128, 4 * NS], F32)
            R2ab = []
            for i in range(2):
                r2t = fin_pool.tile([33, QB], F32R, tag=f"R2_{i}")
                nc.sync.dma_start(r2t[32:33, :], neg1_d[:])
                R2ab.append(r2t)

            H = {"numA": {}, "mom": {}, "Vm": {}, "rz2": {}, "small": {},
                 "u0": {}, "Wt": {}}

            def emit_loads_A(s):
                ga = GA[s]
                na = ga * KG
                kA_t = keys_pool.tile([34, GAmax * KG], F32R, tag="kA")
                nc.sync.dma_start(kA_t[:, :na], kA_d[:, offa[s]:offa[s] + na])
                vA_t = vals_pool.tile([128, GAmax * 2 * 33], F32R, tag="vA")
                nc.sync.dma_start(
                    vA_t[:, :ga * 66],
                    vAr_d[:, (offa[s] // 128) * 33:
                          ((offa[s] + na) // 128) * 33])
                qA_t = qrys_pool.tile([34, QB], F32R, tag="qA")
                nc.sync.dma_start(qA_t[:], qA_d[:, s * QB:(s + 1) * QB])
                nb = GB[s] * KG
                kB_t = keys_pool.tile([33, GBmax * KG], F32R, tag="kB")
                nc.sync.dma_start(kB_t[:, :nb], kB_d[:, offb[s]:offb[s] + nb])
                return kA_t, vA_t, qA_t, kB_t

            def emit_A(s, kA_t, vA_t, qA_t):
                ga = GA[s]
                numA = na_psum.tile([33, QB], F32, tag="numA")
                H["numA"][s] = numA
                for g in range(ga):
                    sc = sc_psum.tile([128, 2 * QB], F32, tag="sc")
                    P = pa_pool.tile([128, 2 * QB], F32R, tag="pa")
                    for h in range(2):
                        ch = 2 * g + h
                        nc.tensor.matmul(
                            sc[:, h * QB:(h + 1) * QB],
                            kA_t[:, ch * 128:(ch + 1) * 128], qA_t[:],
                            start=True, stop=True)
                    nc.scalar.activation(P[:], sc[:], AFT.Exp)
                    for h in range(2):
                        ch = 2 * g + h
                        nc.tensor.matmul(
                            numA[:],
                            vA_t[:, ch * 33:(ch + 1) * 33],
                            P[:, h * QB:(h + 1) * QB],
                            start=(g == 0 and h == 0),
                            stop=(g == ga - 1 and h == 1))

            def emit_epiA(s):
                numA = H["numA"][s]
                rz0 = epi_pool.tile([1, QB], F32, tag="rz0")
                nc.vector.reciprocal(rz0[:], numA[32:33, :])
                rb = epi_pool.tile([32, QB], F32, tag="rb")
                nc.gpsimd.partition_broadcast(rb[:], rz0[:])
                R2 = R2ab[s % 2]
                nc.vector.tensor_mul(R2[0:32, :], numA[0:32, :], rb[:])

            def emit_B(s, kB_t):
                gb = GB[s]
                R2 = R2ab[s % 2]
                mom = mo_psum.tile([128, QB], F32, tag="mom")
                H["mom"][s] = mom
                for g in range(gb):
                    sc = sc_psum.tile([128, 2 * QB], F32, tag="sc")
                    P2 = pa_pool.tile([128, 2 * QB], F32R, tag="pa")
                    for h in range(2):
                        ch = 2 * g + h
                        nc.tensor.matmul(
                            sc[:, h * QB:(h + 1) * QB],
                            kB_t[:, ch * 128:(ch + 1) * 128], R2[:],
                            start=True, stop=True)
                    nc.scalar.activation(P2[:], sc[:], AFT.Exp)
                    for h in range(2):
                        m = 2 * g + h
                        nc.tensor.matmul(
                            mom[0:97, :],
                            mql[:, m * 97:(m + 1) * 97],
                            P2[:, h * QB:(h + 1) * QB],
                            start=(g == 0 and h == 0),
                            stop=(g == gb - 1 and h == 1))

            def emit_epiB_copy(s):
                # iteration-top: free the mom bank ASAP
                mom = H["mom"][s]
                Vm = epi_pool.tile([97, QB], F32, tag="Vm")
                nc.vector.tensor_copy(Vm[:], mom[0:97, :])
                rz2 = epi_pool.tile([1, QB], F32, tag="rz2")
                nc.vector.reciprocal(rz2[:], mom[96:97, :])
                H["Vm"][s] = Vm
                H["rz2"][s] = rz2

            def emit_epiB_mid(s):
                Vm, rz2 = H["Vm"][s], H["rz2"][s]
                small = sm_psum.tile([33, QB], F32, tag="small")
                H["small"][s] = small
                nc.tensor.matmul(small[0:1, :], w1[:, 0:1], Vm[0:96, :],
                                 start=True, stop=True)
                u0 = epi_pool.tile([1, QB], F32, tag="u0")
                nc.vector.tensor_mul(u0[:], small[0:1, :], rz2[:])
                H["u0"][s] = u0
                ub = epi_pool.tile([96, QB], F32, tag="ub")
                nc.gpsimd.partition_broadcast(ub[:], u0[:])
                dt_ = epi_pool.tile([96, QB], F32, tag="dt")
                nc.vector.tensor_scalar_sub(dt_[:], ub[:], coef[:, 0:1])
                tt = epi_pool.tile([96, QB], F32, tag="tt")
                nc.vector.tensor_scalar(tt[:], dt_[:], coef[:, 1:2],
                                        coef[:, 2:3], AluOpType.mult,
                                        AluOpType.add)
                q2 = epi_pool.tile([96, QB], F32, tag="q2")
                nc.vector.tensor_mul(q2[:], tt[:], dt_[:])
                Wt = epi_pool.tile([96, QB], F32, tag="Wt")
                nc.vector.scalar_tensor_tensor(Wt[:], q2[:], coef[:, 3:4],
                                               Vm[0:96, :], AluOpType.add,
                                               AluOpType.mult)
                H["Wt"][s] = Wt

            def emit_epiB_tail(s):
                small, Wt, rz2 = H["small"][s], H["Wt"][s], H["rz2"][s]
                u0 = H["u0"][s]
                nc.tensor.matmul(small[32:33, :], w1[0:96, 1:2], Wt[:],
                                 start=True, stop=True)
                sstd = epi_pool.tile([1, QB], F32, tag="sstd")
                nc.vector.tensor_mul(sstd[:], small[32:33, :], rz2[:])
                for c4 in range(4):
                    nc.sync.dma_start(
                        stats_u[:, s * 4 + c4:s * 4 + c4 + 1],
                        u0[0:1, c4 * 128:(c4 + 1) * 128])
                    nc.sync.dma_start(
                        stats_s[:, s * 4 + c4:s * 4 + c4 + 1],
                        sstd[0:1, c4 * 128:(c4 + 1) * 128])

            # ---- software-pipelined main loop
            tiles0 = emit_loads_A(0)
            emit_A(0, tiles0[0], tiles0[1], tiles0[2])
            emit_epiA(0)
            kB_cur = tiles0[3]
            for s in range(NS):
                if s >= 1:
                    emit_epiB_copy(s - 1)
                if s + 1 < NS:
                    tiles = emit_loads_A(s + 1)
                    emit_A(s + 1, tiles[0], tiles[1], tiles[2])
                    emit_epiA(s + 1)
                    kB_next = tiles[3]
                else:
                    kB_next = None
                emit_B(s, kB_cur)
                kB_cur = kB_next
                if s >= 1:
                    emit_epiB_mid(s - 1)
                if s >= 2:
                    emit_epiB_tail(s - 2)
            emit_epiB_copy(NS - 1)
            emit_epiB_mid(NS - 1)
            if NS >= 2:
                emit_epiB_tail(NS - 2)
            emit_epiB_tail(NS - 1)

            # ---- final: li = (i-u)^2/std + 0.005*ln(std), masked sum
            W = 4 * NS
            stdc = fin_pool.tile([128, W], F32)
            nc.vector.tensor_scalar_max(stdc[:], stats_s[:], STD_FLOOR)
            rstd = fin_pool.tile([128, W], F32)
            nc.vector.reciprocal(rstd[:], stdc[:])
            delta = fin_pool.tile([128, W], F32)
            nc.vector.tensor_sub(delta[:], qidx[:], stats_u[:])
            d2 = fin_pool.tile([128, W], F32)
            nc.vector.tensor_mul(d2[:], delta[:], delta[:])
            t1 = fin_pool.tile([128, W], F32)
            nc.vector.tensor_mul(t1[:], d2[:], rstd[:])
            lg = fin_pool.tile([128, W], F32)
            nc.scalar.activation(lg[:], stdc[:], AFT.Ln)
            lgs = fin_pool.tile([128, W], F32)
            nc.vector.tensor_scalar_mul(lgs[:], lg[:], 0.5 * PENALTY)
            li = fin_pool.tile([128, W], F32)
            nc.vector.tensor_add(li[:], t1[:], lgs[:])
            lim = fin_pool.tile([128, W], F32)
            nc.vector.tensor_mul(lim[:], li[:], qmask[:])
            rowsum = fin_pool.tile([128, 1], F32)
            nc.vector.reduce_sum(rowsum[:], lim[:],
                                 axis=mybir.AxisListType.X)
            tot = sm_psum.tile([33, QB], F32, tag="small")
            nc.tensor.matmul(tot[0:1, 0:1], rowsum[:], ones[:],
                             start=True, stop=True)
            osb = fin_pool.tile([1, 1], F32)
            nc.vector.tensor_copy(osb[:], tot[0:1, 0:1])
            nc.sync.dma_start(out_d[:], osb[:])

    nc.compile()
    return nc


def kernel(seq, src_len, combinations):
    from concourse.bass_utils import run_bass_kernel_spmd

    plan, cores = pack(seq, src_len, combinations)
    nc = build_program(plan)
    in_maps = [{k: ci[k] for k in IN_KEYS} for ci in cores]
    res = run_bass_kernel_spmd(nc, in_maps, list(range(NCORES)))
    tot = np.float32(0.0)
    for c in range(NCORES):
        tot += np.float32(res.results[c]["out"][0, 0])
    n_pairs = np.asarray(combinations).shape[0]
    return np.float32(tot / np.float32(n_pairs))


# revision 9
# speedup vs baseline: 4.8546x; 1.6248x over previous
"""Trainium2 Bass kernel for nn_Cycle_Consistency_Loss (soft-DTW-style
cycle loss). Self-contained: host-side packing + SPMD Bass program on 8
NeuronCores + host reduction.

Math (per pair (a,b), both directions; x = seq[q], y = seq[k], lens = src_len//4):
  alpha = softmax_j(-|x_i-y_j|^2) over valid j -> snn = alpha @ y
  beta  = softmax_k(-|snn_i-x_k|^2) over valid k
  u = E_beta[k], std = E_beta[(k-u)^2]
  li = (i-u)^2/std + 0.005*ln(std), summed over valid i; total / n_pairs.

Kernel decomposition: work items = 512-query blocks of each direction.
Per item, scores are computed transposed ([keys->partitions, queries->free])
via augmented matmuls so softmax denominators reduce over partitions on the
PE. Hot matmuls run in float32r (full PE rate). Pass B computes, per
128-key chunk m, chunk-centered index moments [Zc, M1c, M2c] on the PE via
a sparse constant operand (zero-padded columns accumulate rows 3m..3m+2 of
one [97,512] PSUM tile; column 96 accumulates total Z for free); variance
is recombined per integer chunk center (avoids E[k^2]-u^2 cancellation).
The program is software-pipelined: pass A of step s+1 is issued before
pass B of step s, and the pass-B epilogue is staggered over the next two
iterations, so the PE never waits on the DVE/GpSimd epilogue chains.
Items are packed into octets by (ga,gb) type to minimize per-step padding.
"""
import sys
import numpy as np

sys.path.insert(0, "/opt/trn_rl_repo")

QB = 512          # query block = matmul free dim = one PSUM bank of fp32
KG = 256          # key group (2 chunks of 128 partitions)
NCORES = 8
PENALTY = 0.01
BIG = 1.0e30
STD_FLOOR = 1.0e-6


def _ceil(a, b):
    return -(-a // b)


class _Item:
    __slots__ = ("qi", "ki", "Lq", "Lk", "qb", "ga", "gb", "dummy")

    def __init__(self, qi, ki, Lq, Lk, qb):
        self.qi, self.ki, self.Lq, self.Lk, self.qb = qi, ki, Lq, Lk, qb
        self.ga = _ceil(Lk, KG)
        self.gb = _ceil(Lq, KG)
        self.dummy = False


class _Dummy:
    qi = ki = Lq = Lk = qb = 0
    ga = gb = 0
    dummy = True


def _pack_octets(items):
    """Pack items into octets (one per step, one item per core) minimizing
    sum over steps of (max ga + max gb)."""
    from collections import defaultdict

    groups = defaultdict(list)
    for it in items:
        groups[(it.ga, it.gb)].append(it)
    octets = []
    rest = []
    for key in sorted(groups, key=lambda k: -(k[0] + k[1])):
        lst = groups[key]
        while len(lst) >= NCORES:
            octets.append(lst[:NCORES])
            lst = lst[NCORES:]
        rest.extend(lst)
    rest.sort(key=lambda it: -(it.ga + it.gb))
    while rest:
        cur = [rest.pop(0)]
        cga, cgb = cur[0].ga, cur[0].gb
        while len(cur) < NCORES and rest:
            best_i, best_key = None, None
            for i, it in enumerate(rest):
                inc = max(it.ga - cga, 0) + max(it.gb - cgb, 0)
                key = (inc, -(it.ga + it.gb))
                if best_key is None or key < best_key:
                    best_key, best_i = key, i
            it = rest.pop(best_i)
            cur.append(it)
            cga = max(cga, it.ga)
            cgb = max(cgb, it.gb)
        octets.append(cur)
    for o in octets:
        while len(o) < NCORES:
            o.append(_Dummy())

    def cost(o):
        return (max(max(it.ga for it in o), 1)
                + max(max(it.gb for it in o), 1))

    for _ in range(40):
        improved = False
        for i in range(len(octets)):
            for j in range(i + 1, len(octets)):
                oi, oj = octets[i], octets[j]
                c0 = cost(oi) + cost(oj)
                for a in range(NCORES):
                    for b in range(NCORES):
                        oi[a], oj[b] = oj[b], oi[a]
                        c1 = cost(oi) + cost(oj)
                        if c1 < c0:
                            c0 = c1
                            improved = True
                        else:
                            oi[a], oj[b] = oj[b], oi[a]
        if not improved:
            break
    octets.sort(key=cost, reverse=True)
    return octets


def pack(seq, src_len, combinations):
    """Build the step plan and per-core input arrays.

    Per-core inputs (fp32 bytes; hot matmul operands declared float32r):
      kA  [34, CA]   pass-A key operand rows [yT; y2; 1] (masked keys y2=BIG)
      vAr [128, CA//128*33]  pass-A values, pre-swizzled so the on-chip
                     [128, 2GA, 33] tile loads with contiguous per-partition
                     rows: vAr[p, g*33+d] = vA[g*128+p, d], vA = [y | 1]
      qA  [34, QB*NS] pass-A query operand rows [2xT; -1; -x2]
      kB  [33, CB]   pass-B key operand rows [2xT; x2] (masked keys x2=BIG)
      mql [128, 32*97] sparse moment lhsT: block m has cols [3m,3m+1,3m+2]
                     = [1, r, r^2] (r = p-64, integer-exact), col 96 = 1
                     (accumulates total Z at out row 96); all other cols 0
      w1  [96, 2]    col0: S1 weights (c_m on rows 3m, 1 on rows 3m+1);
                     col1: ones (stdZ reduce)
      coef [96, 4]   col0 c_m (chunk centers, all 3 rows), col1 a2 (1 on 3m),
                     col2 a1 (-2 on 3m+1), col3 a0 (1 on 3m+2)
      ones [128, 1]; neg1 [1, QB]
      qidx/qmask [128, 4*NS] absolute query index / valid mask per B-slot
    """
    seq = np.asarray(seq, np.float32)
    lens = (np.asarray(src_len).astype(np.int64) // 4).astype(np.int64)
    comb = np.asarray(combinations).astype(np.int64)

    items = []
    for a, b in comb:
        for qi, ki in ((a, b), (b, a)):
            Lq, Lk = int(lens[qi]), int(lens[ki])
            if Lq <= 0 or Lk <= 0:
                continue
            for qb in range(_ceil(Lq, QB)):
                items.append(_Item(int(qi), int(ki), Lq, Lk, qb))

    octets = _pack_octets(items)
    NS = len(octets)
    GA = [max(max(it.ga for it in o), 1) for o in octets]
    GB = [max(max(it.gb for it in o), 1) for o in octets]
    CA = sum(GA) * KG
    CB = sum(GB) * KG

    sq2 = np.einsum("btd,btd->bt", seq, seq).astype(np.float32)

    def _split(x, bits=10):
        x = np.asarray(x, np.float32)
        m, e = np.frexp(x)
        h = np.ldexp(np.round(m * (1 << bits)) / (1 << bits), e)
        h = h.astype(np.float32)
        return h, (x - h).astype(np.float32)

    seq_h, seq_l = _split(seq)
    sq2_h, sq2_l = _split(sq2)

    # constants (same for all cores); integer chunk centers c_m = 128m + 64
    p = np.arange(128, dtype=np.float32)
    r = p - 64.0
    mql = np.zeros((128, 32 * 97), np.float32)
    for m in range(32):
        mql[:, m * 97 + 3 * m + 0] = 1.0
        mql[:, m * 97 + 3 * m + 1] = r
        mql[:, m * 97 + 3 * m + 2] = r * r
        mql[:, m * 97 + 96] = 1.0
    cm = 128.0 * np.arange(32, dtype=np.float32) + 64.0
    w1 = np.zeros((96, 2), np.float32)
    coef = np.zeros((96, 4), np.float32)
    for m in range(32):
        w1[3 * m + 0, 0] = cm[m]
        w1[3 * m + 1, 0] = 1.0
        coef[3 * m:3 * m + 3, 0] = cm[m]
        coef[3 * m + 0, 1] = 1.0
        coef[3 * m + 1, 2] = -2.0
        coef[3 * m + 2, 3] = 1.0
    w1[:, 1] = 1.0
    ones = np.ones((128, 1), np.float32)
    neg1 = np.full((2, QB), -1.0, np.float32)

    cores = []
    for c in range(NCORES):
        kA = np.zeros((100, CA), np.float32)
        vA = np.zeros((CA, 33), np.float32)
        qA = np.zeros((100, QB * NS), np.float32)
        kB = np.zeros((98, CB), np.float32)
        qidx = np.zeros((128, 4 * NS), np.float32)
        qmask = np.zeros((128, 4 * NS), np.float32)
        offa = 0
        offb = 0
        its = []
        for s in range(NS):
            it = octets[s][c]
            its.append(it)
            na = GA[s] * KG
            nb = GB[s] * KG
            ka = kA[:, offa:offa + na]
            va = vA[offa:offa + na]
            kb = kB[:, offb:offb + nb]
            qa = qA[:, s * QB:(s + 1) * QB]
            if it.dummy:
                ka[98:100, :] = 1.0
                va[:, 32] = 1.0
            else:
                y = seq[it.ki]
                x = seq[it.qi]
                Lk, Lq = it.Lk, it.Lq
                nk = min(Lk, na)
                # kA rows: [y_h; y_h; y_l; y2_h; y2_l; 1; 1]
                ka[0:32, :nk] = seq_h[it.ki, :nk].T
                ka[32:64, :nk] = seq_h[it.ki, :nk].T
                ka[64:96, :nk] = seq_l[it.ki, :nk].T
                ka[96, :nk] = sq2_h[it.ki, :nk]
                ka[97, :nk] = sq2_l[it.ki, :nk]
                ka[98:100, :nk] = 1.0
                ka[96, nk:] = BIG
                ka[98:100, nk:] = 1.0
                va[:nk, 0:32] = y[:nk]
                va[:nk, 32] = 1.0
                q0 = it.qb * QB
                nq = min(Lq - q0, QB)
                # qA rows: [2x_h; 2x_l; 2x_h; -1; -1; -x2_h; -x2_l]
                qa[0:32, :nq] = 2.0 * seq_h[it.qi, q0:q0 + nq].T
                qa[32:64, :nq] = 2.0 * seq_l[it.qi, q0:q0 + nq].T
                qa[64:96, :nq] = 2.0 * seq_h[it.qi, q0:q0 + nq].T
                qa[96:98, :nq] = -1.0
                qa[98, :nq] = -sq2_h[it.qi, q0:q0 + nq]
                qa[99, :nq] = -sq2_l[it.qi, q0:q0 + nq]
                nkb = min(Lq, nb)
                # kB rows: [2x_h; 2x_h; 2x_l; x2_h; x2_l]
                kb[0:32, :nkb] = 2.0 * seq_h[it.qi, :nkb].T
                kb[32:64, :nkb] = 2.0 * seq_h[it.qi, :nkb].T
                kb[64:96, :nkb] = 2.0 * seq_l[it.qi, :nkb].T
                kb[96, :nkb] = sq2_h[it.qi, :nkb]
                kb[97, :nkb] = sq2_l[it.qi, :nkb]
                kb[96, nkb:] = BIG
                for c4 in range(4):
                    ii = q0 + c4 * 128 + np.arange(128)
                    qidx[:, s * 4 + c4] = ii.astype(np.float32)
                    qmask[:, s * 4 + c4] = (ii < Lq).astype(np.float32)
            offa += na
            offb += nb
        vAr = np.ascontiguousarray(
            vA.reshape(CA // 128, 128, 33).transpose(1, 0, 2).reshape(128, -1))
        cores.append(dict(kA=kA, vAr=vAr, qA=qA, kB=kB, mql=mql, w1=w1,
                          coef=coef, ones=ones, neg1=neg1, qidx=qidx,
                          qmask=qmask, items=its))
    plan = dict(NS=NS, GA=GA, GB=GB, CA=CA, CB=CB)
    return plan, cores


IN_KEYS = ("kA", "vAr", "qA", "kB", "mql", "w1", "coef", "ones", "neg1",
           "qidx", "qmask")


def build_program(plan):
    """Build the software-pipelined SPMD Bass program."""
    import concourse.bass as bass
    import concourse.bacc as bacc
    import concourse.mybir as mybir
    import concourse.tile as tile
    from concourse.alu_op_type import AluOpType

    F32 = mybir.dt.float32
    F32R = mybir.dt.float32r
    AFT = mybir.ActivationFunctionType
    NS, GA, GB = plan["NS"], plan["GA"], plan["GB"]
    CA, CB = plan["CA"], plan["CB"]
    GBmax = max(GB)
    GAmax = max(GA)

    nc = bacc.Bacc("TRN2", target_bir_lowering=False, debug=False,
                   num_devices=NCORES)
    kA_d = nc.dram_tensor("kA", [100, CA], F32R, kind="ExternalInput")
    vAr_d = nc.dram_tensor("vAr", [128, (CA // 128) * 33], F32R,
                           kind="ExternalInput")
    qA_d = nc.dram_tensor("qA", [100, QB * NS], F32R, kind="ExternalInput")
    kB_d = nc.dram_tensor("kB", [98, CB], F32R, kind="ExternalInput")
    mql_d = nc.dram_tensor("mql", [128, 32 * 97], F32R, kind="ExternalInput")
    w1_d = nc.dram_tensor("w1", [96, 2], F32, kind="ExternalInput")
    coef_d = nc.dram_tensor("coef", [96, 4], F32, kind="ExternalInput")
    ones_d = nc.dram_tensor("ones", [128, 1], F32, kind="ExternalInput")
    neg1_d = nc.dram_tensor("neg1", [2, QB], F32R, kind="ExternalInput")
    qidx_d = nc.dram_tensor("qidx", [128, 4 * NS], F32, kind="ExternalInput")
    qmask_d = nc.dram_tensor("qmask", [128, 4 * NS], F32,
                             kind="ExternalInput")
    out_d = nc.dram_tensor("out", [1, 1], F32, kind="ExternalOutput")

    offa = [0] * (NS + 1)
    offb = [0] * (NS + 1)
    for s in range(NS):
        offa[s + 1] = offa[s] + GA[s] * KG
        offb[s + 1] = offb[s] + GB[s] * KG

    with tile.TileContext(nc) as tc:
        with (
            tc.tile_pool(name="keys", bufs=2) as keys_pool,
            tc.tile_pool(name="vals", bufs=2) as vals_pool,
            tc.tile_pool(name="qrys", bufs=2) as qrys_pool,
            tc.tile_pool(name="pa", bufs=2) as pa_pool,
            tc.tile_pool(name="epi", bufs=2) as epi_pool,
            tc.tile_pool(name="fin", bufs=1) as fin_pool,
            tc.tile_pool(name="sc_ps", bufs=2, space="PSUM") as sc_psum,
            tc.tile_pool(name="na_ps", bufs=1, space="PSUM") as na_psum,
            tc.tile_pool(name="mo_ps", bufs=1, space="PSUM") as mo_psum,
            tc.tile_pool(name="sm_ps", bufs=2, space="PSUM") as sm_psum,
        ):
            mql = fin_pool.tile([128, 32 * 97], F32R)
            nc.sync.dma_start(mql[:], mql_d[:])
            w1 = fin_pool.tile([96, 2], F32)
            nc.sync.dma_start(w1[:], w1_d[:])
            coef = fin_pool.tile([96, 4], F32)
            nc.sync.dma_start(coef[:], coef_d[:])
            ones = fin_pool.tile([128, 1], F32)
            nc.sync.dma_start(ones[:], ones_d[:])
            qidx = fin_pool.tile([128, 4 * NS], F32)
            nc.sync.dma_start(qidx[:], qidx_d[:])
            qmask = fin_pool.tile([128, 4 * NS], F32)
            nc.sync.dma_start(qmask[:], qmask_d[:])
            stats_u = fin_pool.tile([128, 4 * NS], F32)
            stats_s = fin_pool.tile([128, 4 * NS], F32)
            R2ab = []
            for i in range(2):
                r2t = fin_pool.tile([98, QB], F32R, tag=f"R2_{i}")
                nc.sync.dma_start(r2t[96:98, :], neg1_d[:])
                R2ab.append(r2t)

            H = {"numA": {}, "mom": {}, "Vm": {}, "rz2": {}, "small": {},
                 "u0": {}, "Wt": {}}

            def emit_loads_A(s):
                ga = GA[s]
                na = ga * KG
                kA_t = keys_pool.tile([100, GAmax * KG], F32R, tag="kA")
                nc.sync.dma_start(kA_t[:, :na], kA_d[:, offa[s]:offa[s] + na])
                vA_t = vals_pool.tile([128, GAmax * 2 * 33], F32R, tag="vA")
                nc.sync.dma_start(
                    vA_t[:, :ga * 66],
                    vAr_d[:, (offa[s] // 128) * 33:
                          ((offa[s] + na) // 128) * 33])
                qA_t = qrys_pool.tile([100, QB], F32R, tag="qA")
                nc.sync.dma_start(qA_t[:], qA_d[:, s * QB:(s + 1) * QB])
                nb = GB[s] * KG
                kB_t = keys_pool.tile([98, GBmax * KG], F32R, tag="kB")
                nc.sync.dma_start(kB_t[:, :nb], kB_d[:, offb[s]:offb[s] + nb])
                return kA_t, vA_t, qA_t, kB_t

            def emit_A(s, kA_t, vA_t, qA_t):
                ga = GA[s]
                numA = na_psum.tile([33, QB], F32, tag="numA")
                H["numA"][s] = numA

                def val_mms(g, P):
                    for h in range(2):
                        ch = 2 * g + h
                        nc.tensor.matmul(
                            numA[:],
                            vA_t[:, ch * 33:(ch + 1) * 33],
                            P[:, h * QB:(h + 1) * QB],
                            start=(g == 0 and h == 0),
                            stop=(g == ga - 1 and h == 1))

                prev = None
                for g in range(ga):
                    sc = sc_psum.tile([128, 2 * QB], F32, tag="sc")
                    for h in range(2):
                        ch = 2 * g + h
                        nc.tensor.matmul(
                            sc[:, h * QB:(h + 1) * QB],
                            kA_t[:, ch * 128:(ch + 1) * 128], qA_t[:],
                            start=True, stop=True)
                    P = pa_pool.tile([128, 2 * QB], F32R, tag="pa")
                    nc.scalar.activation(P[:], sc[:], AFT.Exp)
                    if prev is not None:
                        val_mms(*prev)
                    prev = (g, P)
                val_mms(*prev)

            def emit_epiA(s):
                numA = H["numA"][s]
                rz0 = epi_pool.tile([1, QB], F32, tag="rz0")
                nc.vector.reciprocal(rz0[:], numA[32:33, :])
                rb = epi_pool.tile([32, QB], F32, tag="rb")
                nc.gpsimd.partition_broadcast(rb[:], rz0[:])
                R2 = R2ab[s % 2]
                # snn, then Veltkamp split at 10 bits: s_h exact under
                # fp32r rounding, s_l tiny
                sn = epi_pool.tile([32, QB], F32, tag="sn")
                nc.vector.tensor_mul(sn[:], numA[0:32, :], rb[:])
                t2 = epi_pool.tile([32, QB], F32, tag="t2")
                nc.vector.tensor_scalar_mul(t2[:], sn[:], 16385.0)
                u1 = epi_pool.tile([32, QB], F32, tag="u1")
                nc.vector.tensor_sub(u1[:], t2[:], sn[:])
                sh = epi_pool.tile([32, QB], F32, tag="sh")
                nc.vector.tensor_sub(sh[:], t2[:], u1[:])
                nc.vector.tensor_copy(R2[0:32, :], sh[:])
                nc.vector.tensor_copy(R2[64:96, :], sh[:])
                sl = epi_pool.tile([32, QB], F32, tag="sl")
                nc.vector.tensor_sub(sl[:], sn[:], sh[:])
                nc.vector.tensor_copy(R2[32:64, :], sl[:])

            def emit_B(s, kB_t):
                gb = GB[s]
                R2 = R2ab[s % 2]
                mom = mo_psum.tile([128, QB], F32, tag="mom")
                H["mom"][s] = mom
                def mom_mms(g, P2):
                    for h in range(2):
                        m = 2 * g + h
                        nc.tensor.matmul(
                            mom[0:97, :],
                            mql[:, m * 97:(m + 1) * 97],
                            P2[:, h * QB:(h + 1) * QB],
                            start=(g == 0 and h == 0),
                            stop=(g == gb - 1 and h == 1))

                prev = None
                for g in range(gb):
                    sc = sc_psum.tile([128, 2 * QB], F32, tag="sc")
                    for h in range(2):
                        ch = 2 * g + h
                        nc.tensor.matmul(
                            sc[:, h * QB:(h + 1) * QB],
                            kB_t[:, ch * 128:(ch + 1) * 128], R2[:],
                            start=True, stop=True)
                    P2 = pa_pool.tile([128, 2 * QB], F32R, tag="pa")
                    nc.scalar.activation(P2[:], sc[:], AFT.Exp)
                    if prev is not None:
                        mom_mms(*prev)
                    prev = (g, P2)
                mom_mms(*prev)

            def emit_epiB_copy(s):
                # iteration-top: free the mom bank ASAP
                mom = H["mom"][s]
                Vm = epi_pool.tile([97, QB], F32, tag="Vm")
                nc.vector.tensor_copy(Vm[:], mom[0:97, :])
                rz2 = epi_pool.tile([1, QB], F32, tag="rz2")
                nc.vector.reciprocal(rz2[:], mom[96:97, :])
                H["Vm"][s] = Vm
                H["rz2"][s] = rz2

            def emit_epiB_mid(s):
                Vm, rz2 = H["Vm"][s], H["rz2"][s]
                small = sm_psum.tile([33, QB], F32, tag="small")
                H["small"][s] = small
                nc.tensor.matmul(small[0:1, :], w1[:, 0:1], Vm[0:96, :],
                                 start=True, stop=True)
                u0 = epi_pool.tile([1, QB], F32, tag="u0")
                nc.vector.tensor_mul(u0[:], small[0:1, :], rz2[:])
                H["u0"][s] = u0
                ub = epi_pool.tile([96, QB], F32, tag="ub")
                nc.gpsimd.partition_broadcast(ub[:], u0[:])
                dt_ = epi_pool.tile([96, QB], F32, tag="dt")
                nc.vector.tensor_scalar_sub(dt_[:], ub[:], coef[:, 0:1])
                tt = epi_pool.tile([96, QB], F32, tag="tt")
                nc.vector.tensor_scalar(tt[:], dt_[:], coef[:, 1:2],
                                        coef[:, 2:3], AluOpType.mult,
                                        AluOpType.add)
                q2 = epi_pool.tile([96, QB], F32, tag="q2")
                nc.vector.tensor_mul(q2[:], tt[:], dt_[:])
                Wt = epi_pool.tile([96, QB], F32, tag="Wt")
                nc.vector.scalar_tensor_tensor(Wt[:], q2[:], coef[:, 3:4],
                                               Vm[0:96, :], AluOpType.add,
                                               AluOpType.mult)
                H["Wt"][s] = Wt

            def emit_epiB_tail(s):
                small, Wt, rz2 = H["small"][s], H["Wt"][s], H["rz2"][s]
                u0 = H["u0"][s]
                nc.tensor.matmul(small[32:33, :], w1[0:96, 1:2], Wt[:],
                                 start=True, stop=True)
                sstd = epi_pool.tile([1, QB], F32, tag="sstd")
                nc.vector.tensor_mul(sstd[:], small[32:33, :], rz2[:])
                for c4 in range(4):
                    nc.sync.dma_start(
                        stats_u[:, s * 4 + c4:s * 4 + c4 + 1],
                        u0[0:1, c4 * 128:(c4 + 1) * 128])
                    nc.sync.dma_start(
                        stats_s[:, s * 4 + c4:s * 4 + c4 + 1],
                        sstd[0:1, c4 * 128:(c4 + 1) * 128])

            # ---- software-pipelined main loop
            tiles0 = emit_loads_A(0)
            emit_A(0, tiles0[0], tiles0[1], tiles0[2])
            emit_epiA(0)
            kB_cur = tiles0[3]
            for s in range(NS):
                if s >= 1:
                    emit_epiB_copy(s - 1)
                if s + 1 < NS:
                    tiles = emit_loads_A(s + 1)
                    emit_A(s + 1, tiles[0], tiles[1], tiles[2])
                    emit_epiA(s + 1)
                    kB_next = tiles[3]
                else:
                    kB_next = None
                emit_B(s, kB_cur)
                kB_cur = kB_next
                if s >= 1:
                    emit_epiB_mid(s - 1)
                if s >= 2:
                    emit_epiB_tail(s - 2)
            emit_epiB_copy(NS - 1)
            emit_epiB_mid(NS - 1)
            if NS >= 2:
                emit_epiB_tail(NS - 2)
            emit_epiB_tail(NS - 1)

            # ---- final: li = (i-u)^2/std + 0.005*ln(std), masked sum
            W = 4 * NS
            stdc = fin_pool.tile([128, W], F32)
            nc.vector.tensor_scalar_max(stdc[:], stats_s[:], STD_FLOOR)
            rstd = fin_pool.tile([128, W], F32)
            nc.vector.reciprocal(rstd[:], stdc[:])
            delta = fin_pool.tile([128, W], F32)
            nc.vector.tensor_sub(delta[:], qidx[:], stats_u[:])
            d2 = fin_pool.tile([128, W], F32)
            nc.vector.tensor_mul(d2[:], delta[:], delta[:])
            t1 = fin_pool.tile([128, W], F32)
            nc.vector.tensor_mul(t1[:], d2[:], rstd[:])
            lg = fin_pool.tile([128, W], F32)
            nc.scalar.activation(lg[:], stdc[:], AFT.Ln)
            lgs = fin_pool.tile([128, W], F32)
            nc.vector.tensor_scalar_mul(lgs[:], lg[:], 0.5 * PENALTY)
            li = fin_pool.tile([128, W], F32)
            nc.vector.tensor_add(li[:], t1[:], lgs[:])
            lim = fin_pool.tile([128, W], F32)
            nc.vector.tensor_mul(lim[:], li[:], qmask[:])
            rowsum = fin_pool.tile([128, 1], F32)
            nc.vector.reduce_sum(rowsum[:], lim[:],
                                 axis=mybir.AxisListType.X)
            tot = sm_psum.tile([33, QB], F32, tag="small")
            nc.tensor.matmul(tot[0:1, 0:1], rowsum[:], ones[:],
                             start=True, stop=True)
            osb = fin_pool.tile([1, 1], F32)
            nc.vector.tensor_copy(osb[:], tot[0:1, 0:1])
            nc.sync.dma_start(out_d[:], osb[:])

    nc.compile()
    return nc


def kernel(seq, src_len, combinations):
    from concourse.bass_utils import run_bass_kernel_spmd

    plan, cores = pack(seq, src_len, combinations)
    nc = build_program(plan)
    in_maps = [{k: ci[k] for k in IN_KEYS} for ci in cores]
    res = run_bass_kernel_spmd(nc, in_maps, list(range(NCORES)))
    tot = np.float32(0.0)
    for c in range(NCORES):
        tot += np.float32(res.results[c]["out"][0, 0])
    n_pairs = np.asarray(combinations).shape[0]
    return np.float32(tot / np.float32(n_pairs))
